# revision 6
# baseline (speedup 1.0000x reference)
"""Multi-head attention (B=2, S=2048, H=2048, NH=16, HD=128) on 8 trn2 cores.

Sharding: core i -> (batch b = i // 4, head-group g = i % 4, 4 heads each).
Each core computes q/k/v projections for its 4 heads, causal-masked
attention, and a partial output projection against its 512-row slice of
Wo.  The host sums the 4 partial outputs per batch.

Layout strategy (everything K-major so no on-chip transposes are needed):
  - host ships x^T (per batch) in bf16; projections compute q^T/k^T
    [d, t] via lhsT=W, rhs=x^T, and v [T, d] via lhsT=x^T, rhs=Wv.
  - scores^T [T, t] = (k^T).T @ q^T; exp on ACT (no max subtraction --
    scores are O(6) here, exp is safe in fp32); runtime mask applied
    multiplicatively AFTER exp (so softmax denominators stay exact).
  - causal staircase blocks: the score / od matmuls and the exp are
    restricted to the visible query range [delta, TBLK); the invisible
    prefix of the e tile is zeroed by a Pool memset so the denominator
    tree can keep full-width adds.
  - softmax denominators: e tiles accumulate on DVE into an fp32 esum,
    reduced across partitions with one ones-matmul per (head, block);
    o^T [d, t] = v.T @ e accumulates in PSUM; normalized by broadcast
    reciprocal on the way out to SBUF.  The tiny reciprocal bounce DMAs
    ride the (otherwise idle) DVE DMA queue so they never queue behind
    bulk transfers.
  - final: out[t, m] = (o^T).T @ Wo_rows, accumulated over the 4 heads,
    stored as bf16 partials (host upcasts and sums).

The mask is inspected on the host and the kernel is specialized per
128x512 block: skip (all False), full (all True), affine (causal
staircase), or partial (loads the mask tile and multiplies).

Emission is software-pipelined: in query-block tau's slot we emit its
attention heads with the projections of tau+1 and the output-projection
rows of tau-1 as PE filler, so the PE always has independent matmul
work while ACT grinds through the exps.  The last query block's output
rows open their PSUM accumulation with heads 0-2 around the final
head's reciprocal chain so the PE never starves at the end.
"""

import math

import numpy as np
import ml_dtypes

B, S, H, NH, HD = 2, 2048, 2048, 16, 128
N_CORES = 8
GROUPS = 4                # head-groups (cores per batch)
HPC = NH // GROUPS        # heads per core = 4
DPC = HPC * HD            # head dims per core = 512
TBLK = 512                # query-block width (matmul moving dim)
KBLK = 128                # key-block width (matmul contraction dim)
NT = S // TBLK            # 4 query blocks
NK = S // KBLK            # 16 key blocks
HKT = H // 128            # 16 contraction tiles over hidden dim
HKC = 4                   # contraction chunks per DMA (so loads pipeline)

_BF16 = ml_dtypes.bfloat16

_kernel_cache = {}


MODE_FULL, MODE_AFFINE, MODE_LOADMASK = 0, 1, 2


def _runs(blocks):
    """Group the load-mask blocks of one query block into contiguous Tb
    runs so each run loads with a single DMA."""
    runs = []
    for Tb, mode in blocks:
        if mode != MODE_LOADMASK:
            continue
        if runs and runs[-1][-1] == Tb - 1 and len(runs[-1]) < 4:
            runs[-1].append(Tb)
        else:
            runs.append([Tb])
    return runs


def _build(pattern):
    """Compile the SPMD program for a given mask block pattern.

    pattern: tuple over query-block tau of tuples of (Tb, mode) pairs,
    ascending in Tb, listing key blocks that have any visible entry.
    """
    import concourse.bass as bass  # noqa: F401
    import concourse.tile as tile
    from concourse import bacc, mybir

    fp32 = mybir.dt.float32
    bf16 = mybir.dt.bfloat16
    Exp = mybir.ActivationFunctionType.Exp
    inv_sqrt_hd = 1.0 / math.sqrt(HD)

    all_runs = [_runs(blocks) for blocks in pattern]
    max_run_len = max((len(r) for runs in all_runs for r in runs), default=1)
    max_runs = max((len(runs) for runs in all_runs), default=1)

    nc = bacc.Bacc("TRN2", target_bir_lowering=False, debug=False,
                   num_devices=N_CORES)
    xT = nc.dram_tensor("xT", [H, S], bf16, kind="ExternalInput")
    wq = nc.dram_tensor("wq", [H, DPC], bf16, kind="ExternalInput")
    wk = nc.dram_tensor("wk", [H, DPC], bf16, kind="ExternalInput")
    wv = nc.dram_tensor("wv", [H, DPC], bf16, kind="ExternalInput")
    wo = nc.dram_tensor("wo", [DPC, H], bf16, kind="ExternalInput")
    maskT = nc.dram_tensor("maskT", [S, S], bf16, kind="ExternalInput")
    out = nc.dram_tensor("out", [S, H], bf16, kind="ExternalOutput")
    rbc = nc.dram_tensor("rbc", [NT * HPC, TBLK], fp32)  # reciprocal bounce

    n_chunks = HKT // HKC  # 4

    with tile.TileContext(nc) as tc:
        with (
            tc.tile_pool(name="persist", bufs=1) as persist,
            tc.tile_pool(name="xt", bufs=6) as xt_pool,
            tc.tile_pool(name="masks", bufs=max(2 * max_runs, 2)) as mask_pool,
            tc.tile_pool(name="e", bufs=9) as e_pool,
            tc.tile_pool(name="outsb", bufs=4) as out_pool,
            tc.tile_pool(name="esum", bufs=7) as esum_pool,
            tc.tile_pool(name="rp", bufs=2) as r_pool,
            tc.tile_pool(name="Rp", bufs=2) as R_pool,
            tc.tile_pool(name="ps_work", bufs=3, space="PSUM") as ps_work,
            tc.tile_pool(name="ps_score", bufs=3, space="PSUM") as ps_score,
            tc.tile_pool(name="ps_acc", bufs=2, space="PSUM") as ps_acc,
        ):
            # --- persistent SBUF tensors -------------------------------
            # DMA queue discipline: the first-needed chunks (wq c0, xT c0)
            # sit at the HEAD of four different engine queues so the first
            # projection matmul fires as soon as ~768KB lands instead of
            # waiting for the whole 6.5MB startup burst.  vector's queue
            # is otherwise reserved for the tiny reciprocal bounce DMAs
            # (sub-microsecond latency, never behind bulk traffic).
            # Never tensor: its sequencer must stay on the matmul stream.
            WCH = 2  # contraction tiles per weight-load chunk
            w_sbs = {"wq": [None] * (HKT // WCH), "wk": [None] * (HKT // WCH),
                     "wv": [None] * (HKT // WCH)}
            xt0_tiles = [None] * n_chunks

            def _load_w(name, dram, c, eng):
                t = persist.tile([128, WCH, DPC], bf16, tag=f"{name}{c}")
                eng.dma_start(
                    t[:],
                    dram.ap()[c * WCH * 128:(c + 1) * WCH * 128, :]
                    .rearrange("(k p) d -> p k d", p=128))
                w_sbs[name][c] = t

            def _load_xt0(c, eng):
                t = xt_pool.tile([128, HKC, TBLK], bf16, tag="xt")
                eng.dma_start(
                    t[:],
                    xT.ap()[c * HKC * 128:(c + 1) * HKC * 128, 0:TBLK]
                    .rearrange("(k p) t -> p k t", p=128))
                xt0_tiles[c] = t

            # queue heads: wq c0 -> gpsimd, wq c1 -> scalar, xt c0/c1 ->
            # sync; then the rest in consumption order.
            _load_w("wq", wq, 0, nc.gpsimd)
            _load_w("wq", wq, 1, nc.scalar)
            _load_xt0(0, nc.sync)
            _load_w("wq", wq, 2, nc.gpsimd)
            _load_w("wq", wq, 3, nc.scalar)
            _load_xt0(1, nc.sync)
            _load_w("wq", wq, 4, nc.gpsimd)
            _load_w("wq", wq, 5, nc.scalar)
            _load_xt0(2, nc.sync)
            _load_w("wq", wq, 6, nc.gpsimd)
            _load_w("wq", wq, 7, nc.scalar)
            _load_xt0(3, nc.sync)
            for c in range(HKT // WCH):
                _load_w("wk", wk, c, nc.sync if c % 2 == 0 else nc.gpsimd)
            for c in range(HKT // WCH):
                _load_w("wv", wv, c, nc.scalar)
            wo_sb = persist.tile([128, HPC, H], bf16, tag="wo")
            nc.scalar.dma_start(
                wo_sb[:], wo.ap().rearrange("(c p) m -> p c m", p=128))

            qT_sb = persist.tile([128, HPC, S], bf16, tag="qT")
            kT_sb = persist.tile([128, HPC, S], bf16, tag="kT")
            v_sb = persist.tile([128, NK, DPC], bf16, tag="v")
            oT_sb = persist.tile([128, HPC, S], bf16, tag="oT")

            ones_bf_sb = persist.tile([128, 1], bf16, tag="ones_bf")
            nc.vector.memset(ones_bf_sb[:], 1.0)

            def w_chunk(name, hk):
                return w_sbs[name][hk // WCH][:, hk % WCH, :]

            xts = {0: xt0_tiles}

            def emit_xt_load(tau):
                if tau in xts:
                    return
                tsl = slice(tau * TBLK, (tau + 1) * TBLK)
                xts[tau] = []
                for c in range(n_chunks):
                    t = xt_pool.tile([128, HKC, TBLK], bf16, tag="xt")
                    nc.sync.dma_start(
                        t[:],
                        xT.ap()[c * HKC * 128:(c + 1) * HKC * 128, tsl]
                        .rearrange("(k p) t -> p k t", p=128))
                    xts[tau].append(t)

            def xt_chunk(tau, hk):
                return xts[tau][hk // HKC][:, hk % HKC, :]

            def emit_qk_proj(tau, wname, h):
                tsl = slice(tau * TBLK, (tau + 1) * TBLK)
                dst = qT_sb if wname == "wq" else kT_sb
                ps = ps_work.tile([128, TBLK], fp32, tag="ps")
                for hk in range(HKT):
                    nc.tensor.matmul(
                        ps[:],
                        lhsT=w_chunk(wname, hk)[:, h * HD:(h + 1) * HD],
                        rhs=xt_chunk(tau, hk),
                        start=(hk == 0), stop=(hk == HKT - 1))
                nc.vector.tensor_copy(out=dst[:, h, tsl], in_=ps[:])

            def emit_v_proj(tau, tb_local):
                ps = ps_work.tile([128, TBLK], fp32, tag="ps")
                for hk in range(HKT):
                    nc.tensor.matmul(
                        ps[:],
                        lhsT=xt_chunk(tau, hk)[:, tb_local * KBLK:(tb_local + 1) * KBLK],
                        rhs=w_chunk("wv", hk),
                        start=(hk == 0), stop=(hk == HKT - 1))
                nc.vector.tensor_copy(
                    out=v_sb[:, tau * (TBLK // KBLK) + tb_local, :], in_=ps[:])

            mask_tiles = {}

            def emit_mask_loads(tau):
                tsl = slice(tau * TBLK, (tau + 1) * TBLK)
                for run in all_runs[tau]:
                    mt = mask_pool.tile([128, max_run_len, TBLK], bf16,
                                        tag="mask")
                    nc.sync.dma_start(
                        mt[:, :len(run), :],
                        maskT.ap()[run[0] * KBLK:(run[-1] + 1) * KBLK, tsl]
                        .rearrange("(k p) t -> p k t", p=128))
                    for j, Tb in enumerate(run):
                        mask_tiles[(tau, Tb)] = mt[:, j, :]

            def emit_attention_head(tau, h, chunk=3):
                """Generator; yields ('chunk',), ('pre_dn',), ('post_dn',)
                at points where the caller may emit PE filler."""
                tsl = slice(tau * TBLK, (tau + 1) * TBLK)
                blocks = pattern[tau]
                od = ps_acc.tile([128, TBLK], fp32, tag="od")
                tree = []  # (level, tile) stack for streaming bf16 sum tree
                for i, (Tb, mode) in enumerate(blocks):
                    if i and i % chunk == 0:
                        yield "chunk"
                    # causal-staircase blocks only see queries >= delta;
                    # restrict the matmuls/exp to that range (the first
                    # block of the row is always full width, so the PSUM
                    # zero-region accumulate rules are satisfied).
                    w0 = 0
                    if mode == MODE_AFFINE:
                        delta = Tb * KBLK - tau * TBLK
                        if 0 < delta < TBLK and i > 0:
                            w0 = delta
                    sp = ps_score.tile([128, TBLK], fp32, tag="sc")
                    nc.tensor.matmul(
                        sp[:, w0:],
                        lhsT=kT_sb[:, h, Tb * KBLK:(Tb + 1) * KBLK],
                        rhs=qT_sb[:, h, tau * TBLK + w0:(tau + 1) * TBLK],
                        start=True, stop=True)
                    e = e_pool.tile([128, TBLK], bf16, tag="e")
                    if w0:
                        nc.gpsimd.memset(e[:, :w0], 0.0)
                    nc.scalar.activation(out=e[:, w0:], in_=sp[:, w0:],
                                         func=Exp, scale=inv_sqrt_hd)
                    if mode == MODE_AFFINE:
                        # zero entries where t_rel - T_rel < delta
                        # (value = -(delta-w0) + t'_rel - T_rel, keep >= 0)
                        delta = Tb * KBLK - tau * TBLK
                        nc.gpsimd.affine_select(
                            out=e[:, w0:], in_=e[:, w0:],
                            compare_op=mybir.AluOpType.is_ge,
                            fill=0.0, base=-(delta - w0),
                            pattern=[[1, TBLK - w0]], channel_multiplier=-1)
                    elif mode == MODE_LOADMASK:
                        nc.vector.tensor_mul(e[:], e[:], mask_tiles[(tau, Tb)])
                    cur, lvl = e, 0
                    while tree and tree[-1][0] == lvl:
                        _, prev = tree.pop()
                        acc = esum_pool.tile([128, TBLK], bf16, tag="esum")
                        nc.vector.tensor_add(acc[:], prev[:], cur[:])
                        cur, lvl = acc, lvl + 1
                    tree.append((lvl, cur))
                    nc.tensor.matmul(
                        od[:, w0:],
                        lhsT=v_sb[:, Tb, h * HD:(h + 1) * HD],
                        rhs=e[:, w0:],
                        start=(i == 0), stop=(i == len(blocks) - 1))
                yield "pre_dn"
                while len(tree) > 1:
                    _, a = tree.pop()
                    _, b2 = tree.pop()
                    acc = esum_pool.tile([128, TBLK], bf16, tag="esum")
                    nc.vector.tensor_add(acc[:], a[:], b2[:])
                    tree.append((99, acc))
                esum = tree.pop()[1]
                dn = ps_score.tile([1, TBLK], fp32, tag="sc")
                nc.tensor.matmul(dn[:], lhsT=ones_bf_sb[:], rhs=esum[:],
                                 start=True, stop=True)
                yield "post_dn"
                r = r_pool.tile([1, TBLK], fp32, tag="r")
                nc.vector.reciprocal_approx_fast(out=r[:], in_=dn[:])
                R = R_pool.tile([128, TBLK], fp32, tag="R")
                # partition-broadcast via a DRAM bounce (stride-0 partition
                # reads are only legal on DRAM APs); both DMAs ride the
                # lightly-loaded Pool queue so they never wait behind the
                # bulk output stores (measured 3.4us head-of-line on sync).
                idx = tau * HPC + h
                nc.gpsimd.dma_start(out=rbc.ap()[idx:idx + 1, :], in_=r[:])
                bcast_src = bass.AP(
                    tensor=rbc.ap().tensor, offset=idx * TBLK,
                    ap=[[0, 128], [1, TBLK]])
                nc.gpsimd.dma_start(out=R[:], in_=bcast_src)
                nc.vector.tensor_mul(oT_sb[:, h, tsl], od[:], R[:])

            def emit_out_block(tt, mb):
                # one 128x512 block of the final projection
                ps = ps_work.tile([128, TBLK], fp32, tag="ps")
                for h in range(HPC):
                    nc.tensor.matmul(
                        ps[:],
                        lhsT=oT_sb[:, h, tt * 128:(tt + 1) * 128],
                        rhs=wo_sb[:, h, mb * TBLK:(mb + 1) * TBLK],
                        start=(h == 0), stop=(h == HPC - 1))
                osb = out_pool.tile([128, TBLK], bf16, tag="osb")
                nc.vector.tensor_copy(out=osb[:], in_=ps[:])
                nc.sync.dma_start(
                    out.ap()[tt * 128:(tt + 1) * 128,
                             mb * TBLK:(mb + 1) * TBLK],
                    osb[:])

            # ---- emission schedule -----------------------------------
            # output rows of block tau land in slot tau+1 (spreading the
            # stores through the kernel); the final block's rows are
            # handled specially in slot NT with a heads-0..2 PSUM prefill
            # bracketing the last head's reciprocal chain.
            rows_per_tau = TBLK // 128

            # projections for tau=0 run standalone (startup)
            for h in range(HPC):
                emit_qk_proj(0, "wq", h)
            for h in range(HPC):
                emit_qk_proj(0, "wk", h)
            for tb in range(rows_per_tau):
                emit_v_proj(0, tb)

            emit_mask_loads(0)

            # slot-NT prefill state: last-tau out blocks opened with
            # heads 0..2, closed with head 3 after its chain.
            last_rows = list(range((NT - 1) * rows_per_tau, NT * rows_per_tau))
            prefill_units = [(last_rows[0], mb) for mb in range(H // TBLK)]
            prefill_open = []  # (tt, mb, ps_tile)

            def emit_prefill(n):
                for _ in range(n):
                    if not prefill_units:
                        return
                    tt, mb = prefill_units.pop(0)
                    pool = ps_work if len(prefill_open) < 3 else ps_acc
                    ps = pool.tile([128, TBLK], fp32, tag="ps" if pool is ps_work else "od")
                    for h in range(HPC - 1):
                        nc.tensor.matmul(
                            ps[:],
                            lhsT=oT_sb[:, h, tt * 128:(tt + 1) * 128],
                            rhs=wo_sb[:, h, mb * TBLK:(mb + 1) * TBLK],
                            start=(h == 0), stop=False)
                    prefill_open.append((tt, mb, ps))

            for tau in range(NT):
                fillers = []
                if tau + 1 < NT:
                    emit_xt_load(tau + 1)
                    emit_mask_loads(tau + 1)
                    fillers += [lambda h=h, t=tau + 1: emit_qk_proj(t, "wq", h)
                                for h in range(HPC)]
                    fillers += [lambda h=h, t=tau + 1: emit_qk_proj(t, "wk", h)
                                for h in range(HPC)]
                    fillers += [lambda tb=tb, t=tau + 1: emit_v_proj(t, tb)
                                for tb in range(rows_per_tau)]
                if tau >= 1:
                    # output rows of the previous query block
                    for tt in range((tau - 1) * rows_per_tau,
                                    tau * rows_per_tau):
                        fillers += [lambda tt=tt, mb=mb: emit_out_block(tt, mb)
                                    for mb in range(H // TBLK)]
                fill_iter = iter(fillers)

                def fill(n=1):
                    for _ in range(n):
                        f = next(fill_iter, None)
                        if f is None:
                            return
                        f()

                for h in range(HPC):
                    final_head = (tau == NT - 1 and h == HPC - 1)
                    if final_head:
                        # the prefill groups below hold every ps_work
                        # buffer open until their head-3 close at the very
                        # end; any ps_work allocation emitted in between
                        # would head-block the in-order PE queue.  Drain
                        # the regular fillers first (no-op for the causal
                        # pattern -- they are exhausted by now).
                        for f in fill_iter:
                            f()
                    for ev in emit_attention_head(tau, h):
                        if ev == "chunk":
                            fill(1)
                        elif ev == "pre_dn":
                            # keep the PE fed while DVE collapses the
                            # esum tree / runs the reciprocal chain
                            if final_head:
                                emit_prefill(2)
                            else:
                                fill(1)
                        elif ev == "post_dn":
                            if final_head:
                                emit_prefill(2)
                    if not final_head:
                        fill(1)
                for f in fill_iter:
                    f()

            # ---- final query block's output rows ---------------------
            # close the prefilled groups with head 3, then the rest.
            for tt, mb, ps in prefill_open:
                nc.tensor.matmul(
                    ps[:],
                    lhsT=oT_sb[:, HPC - 1, tt * 128:(tt + 1) * 128],
                    rhs=wo_sb[:, HPC - 1, mb * TBLK:(mb + 1) * TBLK],
                    start=False, stop=True)
                osb = out_pool.tile([128, TBLK], bf16, tag="osb")
                nc.vector.tensor_copy(out=osb[:], in_=ps[:])
                nc.sync.dma_start(
                    out.ap()[tt * 128:(tt + 1) * 128,
                             mb * TBLK:(mb + 1) * TBLK],
                    osb[:])
            done = {(tt, mb) for tt, mb, _ in prefill_open}
            for tt in last_rows:
                for mb in range(H // TBLK):
                    if (tt, mb) not in done:
                        emit_out_block(tt, mb)

    nc.compile()
    return nc


def _classify(mask):
    """Per 128x512 block of mask^T: skip / full / affine / partial,
    unioned over batches.  Returns the pattern tuple, or None if some
    row is fully masked (degenerate -- reference gives uniform weights
    there)."""
    if not mask.any(axis=2).all():
        return None
    tr = np.arange(TBLK)[:, None]
    Tr = np.arange(KBLK)[None, :]
    pattern = []
    for tau in range(NT):
        blocks = []
        for Tb in range(NK):
            # block of mask^T[Tb*128:(Tb+1)*128, tau*512:(tau+1)*512]
            # == mask[:, tau*512:(tau+1)*512, Tb*128:(Tb+1)*128]
            blk = mask[:, tau * TBLK:(tau + 1) * TBLK,
                       Tb * KBLK:(Tb + 1) * KBLK]
            if not blk.any():
                continue
            if blk.all():
                blocks.append((Tb, MODE_FULL))
                continue
            # causal staircase? mask[t, T] = (t >= T), i.e.
            # tau*TBLK + tr >= Tb*KBLK + Tr
            stair = (tau * TBLK + tr) >= (Tb * KBLK + Tr)
            if all((blk[b] == stair).all() for b in range(blk.shape[0])):
                blocks.append((Tb, MODE_AFFINE))
            else:
                blocks.append((Tb, MODE_LOADMASK))
        pattern.append(tuple(blocks))
    return tuple(pattern)


def _reference_fallback(x, mask, Wq, Wk, Wv, Wo):
    out = np.empty((B, S, H), np.float32)
    for b in range(B):
        q = (x[b] @ Wq).reshape(S, NH, HD).transpose(1, 0, 2)
        k = (x[b] @ Wk).reshape(S, NH, HD).transpose(1, 0, 2)
        v = (x[b] @ Wv).reshape(S, NH, HD).transpose(1, 0, 2)
        s = np.einsum("htd,hTd->htT", q, k) / np.sqrt(HD)
        s = np.where(mask[b][None], s, -1e10)
        s -= s.max(-1, keepdims=True)
        w = np.exp(s)
        w /= w.sum(-1, keepdims=True)
        o = np.einsum("htT,hTd->htd", w, v)
        out[b] = o.transpose(1, 0, 2).reshape(S, NH * HD) @ Wo
    return out


def kernel(x, mask, Wq, Wk, Wv, Wo):
    x = np.asarray(x, np.float32)
    mask = np.asarray(mask).astype(bool)
    Wq = np.asarray(Wq, np.float32)
    Wk = np.asarray(Wk, np.float32)
    Wv = np.asarray(Wv, np.float32)
    Wo = np.asarray(Wo, np.float32)
    assert x.shape == (B, S, H) and mask.shape == (B, S, S)

    pattern = _classify(mask)
    if pattern is None:
        return _reference_fallback(x, mask, Wq, Wk, Wv, Wo)

    if pattern not in _kernel_cache:
        _kernel_cache[pattern] = _build(pattern)
    nc = _kernel_cache[pattern]

    xT_b = [np.ascontiguousarray(x[b].T).astype(_BF16) for b in range(B)]
    maskT_b = [np.ascontiguousarray(mask[b].T).astype(_BF16) for b in range(B)]
    wq_g = [np.ascontiguousarray(Wq[:, g * DPC:(g + 1) * DPC]).astype(_BF16)
            for g in range(GROUPS)]
    wk_g = [np.ascontiguousarray(Wk[:, g * DPC:(g + 1) * DPC]).astype(_BF16)
            for g in range(GROUPS)]
    wv_g = [np.ascontiguousarray(Wv[:, g * DPC:(g + 1) * DPC]).astype(_BF16)
            for g in range(GROUPS)]
    wo_g = [np.ascontiguousarray(Wo[g * DPC:(g + 1) * DPC, :]).astype(_BF16)
            for g in range(GROUPS)]

    in_maps = []
    for i in range(N_CORES):
        b, g = divmod(i, GROUPS)
        in_maps.append({
            "xT": xT_b[b], "maskT": maskT_b[b],
            "wq": wq_g[g], "wk": wk_g[g], "wv": wv_g[g], "wo": wo_g[g],
        })

    from concourse.bass_utils import run_bass_kernel_spmd
    res = run_bass_kernel_spmd(nc, in_maps, core_ids=list(range(N_CORES)))

    out = np.zeros((B, S, H), np.float32)
    for i in range(N_CORES):
        b = i // GROUPS
        out[b] += res.results[i]["out"].astype(np.float32)
    return out


# revision 10
# speedup vs baseline: 1.0198x; 1.0198x over previous
"""Multi-head attention (B=2, S=2048, H=2048, NH=16, HD=128) on 8 trn2 cores.

Sharding: core i -> (batch b = i // 4, head-group g = i % 4, 4 heads each).
Each core computes q/k/v projections for its 4 heads, causal-masked
attention, and a partial output projection against its 512-row slice of
Wo.  The host sums the 4 partial outputs per batch.

Layout strategy (everything K-major so no on-chip transposes are needed):
  - host ships x^T (per batch) in bf16; projections compute q^T/k^T
    [d, t] via lhsT=W, rhs=x^T, and v [T, d] via lhsT=x^T, rhs=Wv.
  - scores^T [T, t] = (k^T).T @ q^T; exp on ACT (no max subtraction --
    scores are O(6) here, exp is safe in fp32); runtime mask applied
    multiplicatively AFTER exp (so softmax denominators stay exact).
  - causal staircase blocks: the score / od matmuls and the exp are
    restricted to the visible query range [delta, TBLK); the invisible
    prefix of the e tile is zeroed by a Pool memset so the denominator
    tree can keep full-width adds.
  - softmax denominators: e tiles accumulate on DVE into an fp32 esum,
    reduced across partitions with one ones-matmul per (head, block);
    o^T [d, t] = v.T @ e accumulates in PSUM; normalized by broadcast
    reciprocal on the way out to SBUF.  The tiny reciprocal bounce DMAs
    ride the (otherwise idle) DVE DMA queue so they never queue behind
    bulk transfers.
  - final: out[t, m] = (o^T).T @ Wo_rows, accumulated over the 4 heads,
    stored as bf16 partials (host upcasts and sums).

The mask is inspected on the host and the kernel is specialized per
128x512 block: skip (all False), full (all True), affine (causal
staircase), or partial (loads the mask tile and multiplies).

Emission is software-pipelined: in query-block tau's slot we emit its
attention heads with the projections of tau+1 and the output-projection
rows of tau-1 as PE filler, so the PE always has independent matmul
work while ACT grinds through the exps.  The last query block's output
rows open their PSUM accumulation with heads 0-2 around the final
head's reciprocal chain so the PE never starves at the end.
"""

import math

import numpy as np
import ml_dtypes

B, S, H, NH, HD = 2, 2048, 2048, 16, 128
N_CORES = 8
GROUPS = 4                # head-groups (cores per batch)
HPC = NH // GROUPS        # heads per core = 4
DPC = HPC * HD            # head dims per core = 512
TBLK = 512                # query-block width (matmul moving dim)
KBLK = 128                # key-block width (matmul contraction dim)
NT = S // TBLK            # 4 query blocks
NK = S // KBLK            # 16 key blocks
HKT = H // 128            # 16 contraction tiles over hidden dim
HKC = 4                   # contraction chunks per DMA (so loads pipeline)

_BF16 = ml_dtypes.bfloat16

_kernel_cache = {}


MODE_FULL, MODE_AFFINE, MODE_LOADMASK = 0, 1, 2


def _runs(blocks):
    """Group the load-mask blocks of one query block into contiguous Tb
    runs so each run loads with a single DMA."""
    runs = []
    for Tb, mode in blocks:
        if mode != MODE_LOADMASK:
            continue
        if runs and runs[-1][-1] == Tb - 1 and len(runs[-1]) < 4:
            runs[-1].append(Tb)
        else:
            runs.append([Tb])
    return runs


def _build(pattern):
    """Compile the SPMD program for a given mask block pattern.

    pattern: tuple over query-block tau of tuples of (Tb, mode) pairs,
    ascending in Tb, listing key blocks that have any visible entry.
    """
    import concourse.bass as bass  # noqa: F401
    import concourse.tile as tile
    from concourse import bacc, mybir

    fp32 = mybir.dt.float32
    bf16 = mybir.dt.bfloat16
    Exp = mybir.ActivationFunctionType.Exp
    inv_sqrt_hd = 1.0 / math.sqrt(HD)

    all_runs = [_runs(blocks) for blocks in pattern]
    max_run_len = max((len(r) for runs in all_runs for r in runs), default=1)
    max_runs = max((len(runs) for runs in all_runs), default=1)

    nc = bacc.Bacc("TRN2", target_bir_lowering=False, debug=False,
                   num_devices=N_CORES)
    xT = nc.dram_tensor("xT", [H, S], bf16, kind="ExternalInput")
    wq = nc.dram_tensor("wq", [H, DPC], bf16, kind="ExternalInput")
    wk = nc.dram_tensor("wk", [H, DPC], bf16, kind="ExternalInput")
    wv = nc.dram_tensor("wv", [H, DPC], bf16, kind="ExternalInput")
    wo = nc.dram_tensor("wo", [DPC, H], bf16, kind="ExternalInput")
    maskT = nc.dram_tensor("maskT", [S, S], bf16, kind="ExternalInput")
    out = nc.dram_tensor("out", [S, H], bf16, kind="ExternalOutput")
    rbc = nc.dram_tensor("rbc", [NT * HPC, TBLK], fp32)  # reciprocal bounce

    n_chunks = HKT // HKC  # 4

    with tile.TileContext(nc) as tc:
        with (
            tc.tile_pool(name="persist", bufs=1) as persist,
            tc.tile_pool(name="xt", bufs=6) as xt_pool,
            tc.tile_pool(name="masks", bufs=max(2 * max_runs, 2)) as mask_pool,
            tc.tile_pool(name="e", bufs=9) as e_pool,
            tc.tile_pool(name="outsb", bufs=4) as out_pool,
            tc.tile_pool(name="esum", bufs=7) as esum_pool,
            tc.tile_pool(name="rp", bufs=2) as r_pool,
            tc.tile_pool(name="Rp", bufs=2) as R_pool,
            tc.tile_pool(name="ps_work", bufs=3, space="PSUM") as ps_work,
            tc.tile_pool(name="ps_score", bufs=3, space="PSUM") as ps_score,
            tc.tile_pool(name="ps_acc", bufs=2, space="PSUM") as ps_acc,
        ):
            # --- persistent SBUF tensors -------------------------------
            # DMA queue discipline: the first-needed chunks (wq c0, xT c0)
            # sit at the HEAD of four different engine queues so the first
            # projection matmul fires as soon as ~768KB lands instead of
            # waiting for the whole 6.5MB startup burst.
            # Never tensor: its sequencer must stay on the matmul stream.
            WCH = 2  # contraction tiles per weight-load chunk
            w_sbs = {"wq": [None] * (HKT // WCH), "wk": [None] * (HKT // WCH),
                     "wv": [None] * (HKT // WCH)}
            xt0_tiles = [None] * n_chunks

            def _load_w(name, dram, c, eng):
                t = persist.tile([128, WCH, DPC], bf16, tag=f"{name}{c}")
                eng.dma_start(
                    t[:],
                    dram.ap()[c * WCH * 128:(c + 1) * WCH * 128, :]
                    .rearrange("(k p) d -> p k d", p=128))
                w_sbs[name][c] = t

            def _load_xt0(c, eng):
                t = xt_pool.tile([128, HKC, TBLK], bf16, tag="xt")
                eng.dma_start(
                    t[:],
                    xT.ap()[c * HKC * 128:(c + 1) * HKC * 128, 0:TBLK]
                    .rearrange("(k p) t -> p k t", p=128))
                xt0_tiles[c] = t

            # queue heads: wq c0 -> gpsimd, wq c1 -> scalar, xt c0 ->
            # sync; then the rest interleaved in consumption order so the
            # first projection chain streams at DMA arrival rate.
            _load_w("wq", wq, 0, nc.gpsimd)
            _load_w("wq", wq, 1, nc.scalar)
            _load_xt0(0, nc.sync)
            _load_w("wq", wq, 2, nc.gpsimd)
            _load_w("wq", wq, 3, nc.scalar)
            _load_xt0(1, nc.gpsimd)
            _load_xt0(2, nc.sync)
            _load_w("wq", wq, 4, nc.gpsimd)
            _load_w("wq", wq, 5, nc.scalar)
            _load_xt0(3, nc.scalar)
            _load_w("wq", wq, 6, nc.gpsimd)
            _load_w("wq", wq, 7, nc.scalar)
            for c in range(HKT // WCH):
                _load_w("wk", wk, c, nc.sync if c % 2 == 0 else nc.gpsimd)
            for c in range(HKT // WCH):
                _load_w("wv", wv, c, nc.scalar)
            wo_sb = persist.tile([128, HPC, H], bf16, tag="wo")
            nc.scalar.dma_start(
                wo_sb[:], wo.ap().rearrange("(c p) m -> p c m", p=128))

            qT_sb = persist.tile([128, HPC, S], bf16, tag="qT")
            kT_sb = persist.tile([128, HPC, S], bf16, tag="kT")
            v_sb = persist.tile([128, NK, DPC], bf16, tag="v")
            oT_sb = persist.tile([128, HPC, S], bf16, tag="oT")

            ones_bf_sb = persist.tile([128, 1], bf16, tag="ones_bf")
            nc.vector.memset(ones_bf_sb[:], 1.0)

            def w_chunk(name, hk):
                return w_sbs[name][hk // WCH][:, hk % WCH, :]

            xts = {0: xt0_tiles}

            def emit_xt_load(tau):
                if tau in xts:
                    return
                tsl = slice(tau * TBLK, (tau + 1) * TBLK)
                xts[tau] = []
                for c in range(n_chunks):
                    t = xt_pool.tile([128, HKC, TBLK], bf16, tag="xt")
                    nc.sync.dma_start(
                        t[:],
                        xT.ap()[c * HKC * 128:(c + 1) * HKC * 128, tsl]
                        .rearrange("(k p) t -> p k t", p=128))
                    xts[tau].append(t)

            def xt_chunk(tau, hk):
                return xts[tau][hk // HKC][:, hk % HKC, :]

            def emit_qk_proj(tau, wname, h):
                tsl = slice(tau * TBLK, (tau + 1) * TBLK)
                dst = qT_sb if wname == "wq" else kT_sb
                ps = ps_work.tile([128, TBLK], fp32, tag="ps")
                for hk in range(HKT):
                    nc.tensor.matmul(
                        ps[:],
                        lhsT=w_chunk(wname, hk)[:, h * HD:(h + 1) * HD],
                        rhs=xt_chunk(tau, hk),
                        start=(hk == 0), stop=(hk == HKT - 1))
                nc.vector.tensor_copy(out=dst[:, h, tsl], in_=ps[:])

            def emit_v_proj(tau, tb_local):
                ps = ps_work.tile([128, TBLK], fp32, tag="ps")
                for hk in range(HKT):
                    nc.tensor.matmul(
                        ps[:],
                        lhsT=xt_chunk(tau, hk)[:, tb_local * KBLK:(tb_local + 1) * KBLK],
                        rhs=w_chunk("wv", hk),
                        start=(hk == 0), stop=(hk == HKT - 1))
                nc.vector.tensor_copy(
                    out=v_sb[:, tau * (TBLK // KBLK) + tb_local, :], in_=ps[:])

            mask_tiles = {}

            def emit_mask_loads(tau):
                tsl = slice(tau * TBLK, (tau + 1) * TBLK)
                for run in all_runs[tau]:
                    mt = mask_pool.tile([128, max_run_len, TBLK], bf16,
                                        tag="mask")
                    nc.sync.dma_start(
                        mt[:, :len(run), :],
                        maskT.ap()[run[0] * KBLK:(run[-1] + 1) * KBLK, tsl]
                        .rearrange("(k p) t -> p k t", p=128))
                    for j, Tb in enumerate(run):
                        mask_tiles[(tau, Tb)] = mt[:, j, :]

            def emit_attention_head(tau, h, chunk=3):
                """Generator; yields ('chunk',), ('pre_dn',), ('post_dn',)
                at points where the caller may emit PE filler."""
                tsl = slice(tau * TBLK, (tau + 1) * TBLK)
                blocks = pattern[tau]
                od = ps_acc.tile([128, TBLK], fp32, tag="od")
                tree = []  # (level, tile) stack for streaming bf16 sum tree
                for i, (Tb, mode) in enumerate(blocks):
                    if i and i % chunk == 0:
                        yield "chunk"
                    # causal-staircase blocks only see queries >= delta;
                    # restrict the matmuls/exp to that range (the first
                    # block of the row is always full width, so the PSUM
                    # zero-region accumulate rules are satisfied).
                    w0 = 0
                    if mode == MODE_AFFINE:
                        delta = Tb * KBLK - tau * TBLK
                        if 0 < delta < TBLK and i > 0:
                            w0 = delta
                    sp = ps_score.tile([128, TBLK], fp32, tag="sc")
                    nc.tensor.matmul(
                        sp[:, w0:],
                        lhsT=kT_sb[:, h, Tb * KBLK:(Tb + 1) * KBLK],
                        rhs=qT_sb[:, h, tau * TBLK + w0:(tau + 1) * TBLK],
                        start=True, stop=True)
                    e = e_pool.tile([128, TBLK], bf16, tag="e")
                    if w0:
                        nc.gpsimd.memset(e[:, :w0], 0.0)
                    nc.scalar.activation(out=e[:, w0:], in_=sp[:, w0:],
                                         func=Exp, scale=inv_sqrt_hd)
                    if mode == MODE_AFFINE:
                        # zero entries where t_rel - T_rel < delta
                        # (value = -(delta-w0) + t'_rel - T_rel, keep >= 0)
                        delta = Tb * KBLK - tau * TBLK
                        nc.gpsimd.affine_select(
                            out=e[:, w0:], in_=e[:, w0:],
                            compare_op=mybir.AluOpType.is_ge,
                            fill=0.0, base=-(delta - w0),
                            pattern=[[1, TBLK - w0]], channel_multiplier=-1)
                    elif mode == MODE_LOADMASK:
                        nc.vector.tensor_mul(e[:], e[:], mask_tiles[(tau, Tb)])
                    cur, lvl = e, 0
                    while tree and tree[-1][0] == lvl:
                        _, prev = tree.pop()
                        acc = esum_pool.tile([128, TBLK], bf16, tag="esum")
                        nc.vector.tensor_add(acc[:], prev[:], cur[:])
                        cur, lvl = acc, lvl + 1
                    tree.append((lvl, cur))
                    nc.tensor.matmul(
                        od[:, w0:],
                        lhsT=v_sb[:, Tb, h * HD:(h + 1) * HD],
                        rhs=e[:, w0:],
                        start=(i == 0), stop=(i == len(blocks) - 1))
                yield "pre_dn"
                while len(tree) > 1:
                    _, a = tree.pop()
                    _, b2 = tree.pop()
                    acc = esum_pool.tile([128, TBLK], bf16, tag="esum")
                    nc.vector.tensor_add(acc[:], a[:], b2[:])
                    tree.append((99, acc))
                esum = tree.pop()[1]
                dn = ps_score.tile([1, TBLK], fp32, tag="sc")
                nc.tensor.matmul(dn[:], lhsT=ones_bf_sb[:], rhs=esum[:],
                                 start=True, stop=True)
                yield "post_dn"
                r = r_pool.tile([1, TBLK], fp32, tag="r")
                nc.vector.reciprocal_approx_fast(out=r[:], in_=dn[:])
                R = R_pool.tile([128, TBLK], fp32, tag="R")
                # partition-broadcast via a DRAM bounce (stride-0 partition
                # reads are only legal on DRAM APs) on the sync hardware
                # DMA queue (gpsimd's software DGE is slower for these).
                idx = tau * HPC + h
                nc.sync.dma_start(out=rbc.ap()[idx:idx + 1, :], in_=r[:])
                bcast_src = bass.AP(
                    tensor=rbc.ap().tensor, offset=idx * TBLK,
                    ap=[[0, 128], [1, TBLK]])
                nc.sync.dma_start(out=R[:], in_=bcast_src)
                nc.vector.tensor_mul(oT_sb[:, h, tsl], od[:], R[:])

            def emit_out_block(tt, mb):
                # one 128x512 block of the final projection
                ps = ps_work.tile([128, TBLK], fp32, tag="ps")
                for h in range(HPC):
                    nc.tensor.matmul(
                        ps[:],
                        lhsT=oT_sb[:, h, tt * 128:(tt + 1) * 128],
                        rhs=wo_sb[:, h, mb * TBLK:(mb + 1) * TBLK],
                        start=(h == 0), stop=(h == HPC - 1))
                osb = out_pool.tile([128, TBLK], bf16, tag="osb")
                nc.vector.tensor_copy(out=osb[:], in_=ps[:])
                nc.sync.dma_start(
                    out.ap()[tt * 128:(tt + 1) * 128,
                             mb * TBLK:(mb + 1) * TBLK],
                    osb[:])

            # ---- emission schedule -----------------------------------
            # output rows of block tau land in slot tau+1 (spreading the
            # stores through the kernel); the final block's rows are
            # handled specially in slot NT with a heads-0..2 PSUM prefill
            # bracketing the last head's reciprocal chain.
            rows_per_tau = TBLK // 128

            # projections for tau=0 run standalone (startup)
            for h in range(HPC):
                emit_qk_proj(0, "wq", h)
            for h in range(HPC):
                emit_qk_proj(0, "wk", h)
            for tb in range(rows_per_tau):
                emit_v_proj(0, tb)

            emit_mask_loads(0)

            # slot-NT prefill state: last-tau out blocks opened with
            # heads 0..2, closed with head 3 after its chain.
            last_rows = list(range((NT - 1) * rows_per_tau, NT * rows_per_tau))
            prefill_units = [(last_rows[0], mb) for mb in range(H // TBLK)]
            prefill_open = []  # (tt, mb, ps_tile)

            def emit_prefill(n):
                for _ in range(n):
                    if not prefill_units:
                        return
                    tt, mb = prefill_units.pop(0)
                    pool = ps_work if len(prefill_open) < 3 else ps_acc
                    ps = pool.tile([128, TBLK], fp32, tag="ps" if pool is ps_work else "od")
                    for h in range(HPC - 1):
                        nc.tensor.matmul(
                            ps[:],
                            lhsT=oT_sb[:, h, tt * 128:(tt + 1) * 128],
                            rhs=wo_sb[:, h, mb * TBLK:(mb + 1) * TBLK],
                            start=(h == 0), stop=False)
                    prefill_open.append((tt, mb, ps))

            from collections import deque

            for tau in range(NT):
                proj_units, row_units = [], []
                if tau + 1 < NT:
                    emit_xt_load(tau + 1)
                    emit_mask_loads(tau + 1)
                    proj_units += [lambda h=h, t=tau + 1:
                                   emit_qk_proj(t, "wq", h)
                                   for h in range(HPC)]
                    proj_units += [lambda h=h, t=tau + 1:
                                   emit_qk_proj(t, "wk", h)
                                   for h in range(HPC)]
                    proj_units += [lambda tb=tb, t=tau + 1:
                                   emit_v_proj(t, tb)
                                   for tb in range(rows_per_tau)]
                if tau >= 1:
                    # output rows of the previous query block
                    for tt in range((tau - 1) * rows_per_tau,
                                    tau * rows_per_tau):
                        row_units += [lambda tt=tt, mb=mb:
                                      emit_out_block(tt, mb)
                                      for mb in range(H // TBLK)]
                # reservations: short row units are preferred for the
                # pre-dn / between-head edges (they stall the chain less
                # than a 16-matmul projection chain); the final head of
                # the last slot gets a dedicated 6-unit window to hide
                # its reciprocal chain.
                final_window = deque()
                if tau == NT - 1 and len(row_units) >= 6:
                    final_window = deque(row_units[-6:])
                    row_units = row_units[:-6]
                n_edge = 2 * HPC - (2 if tau == NT - 1 else 0)
                edge = deque(row_units[:n_edge])
                rest = row_units[n_edge:]
                if len(edge) < n_edge:
                    take = n_edge - len(edge)
                    edge.extend(proj_units[:take])
                    proj_units = proj_units[take:]
                bulk = deque(proj_units + rest)

                def fill(dq, alt, n=1):
                    for _ in range(n):
                        if dq:
                            dq.popleft()()
                        elif alt:
                            alt.popleft()()

                for h in range(HPC):
                    final_head = (tau == NT - 1 and h == HPC - 1)
                    if final_head:
                        # the prefill groups below hold every ps_work
                        # buffer open until their head-3 close at the
                        # very end; any ps_work allocation emitted in
                        # between would head-block the in-order PE queue.
                        # Drain everything but the reserved window first.
                        while bulk or edge:
                            fill(bulk, edge)
                    for ev in emit_attention_head(tau, h):
                        if ev == "chunk":
                            fill(bulk, None)
                        elif ev == "pre_dn":
                            # keep the PE fed while DVE collapses the
                            # esum tree / runs the reciprocal chain
                            if final_head:
                                fill(final_window, None, 2)
                            else:
                                fill(edge, bulk)
                        elif ev == "post_dn":
                            if final_head:
                                fill(final_window, None, 4)
                                emit_prefill(4)
                    if not final_head:
                        fill(edge, bulk)
                while bulk or edge:
                    fill(bulk, edge)

            # ---- final query block's output rows ---------------------
            # close the prefilled groups with head 3, then the rest.
            for tt, mb, ps in prefill_open:
                nc.tensor.matmul(
                    ps[:],
                    lhsT=oT_sb[:, HPC - 1, tt * 128:(tt + 1) * 128],
                    rhs=wo_sb[:, HPC - 1, mb * TBLK:(mb + 1) * TBLK],
                    start=False, stop=True)
                osb = out_pool.tile([128, TBLK], bf16, tag="osb")
                nc.vector.tensor_copy(out=osb[:], in_=ps[:])
                nc.sync.dma_start(
                    out.ap()[tt * 128:(tt + 1) * 128,
                             mb * TBLK:(mb + 1) * TBLK],
                    osb[:])
            done = {(tt, mb) for tt, mb, _ in prefill_open}
            for tt in last_rows:
                for mb in range(H // TBLK):
                    if (tt, mb) not in done:
                        emit_out_block(tt, mb)

    nc.compile()
    return nc


def _classify(mask):
    """Per 128x512 block of mask^T: skip / full / affine / partial,
    unioned over batches.  Returns the pattern tuple, or None if some
    row is fully masked (degenerate -- reference gives uniform weights
    there)."""
    if not mask.any(axis=2).all():
        return None
    tr = np.arange(TBLK)[:, None]
    Tr = np.arange(KBLK)[None, :]
    pattern = []
    for tau in range(NT):
        blocks = []
        for Tb in range(NK):
            # block of mask^T[Tb*128:(Tb+1)*128, tau*512:(tau+1)*512]
            # == mask[:, tau*512:(tau+1)*512, Tb*128:(Tb+1)*128]
            blk = mask[:, tau * TBLK:(tau + 1) * TBLK,
                       Tb * KBLK:(Tb + 1) * KBLK]
            if not blk.any():
                continue
            if blk.all():
                blocks.append((Tb, MODE_FULL))
                continue
            # causal staircase? mask[t, T] = (t >= T), i.e.
            # tau*TBLK + tr >= Tb*KBLK + Tr
            stair = (tau * TBLK + tr) >= (Tb * KBLK + Tr)
            if all((blk[b] == stair).all() for b in range(blk.shape[0])):
                blocks.append((Tb, MODE_AFFINE))
            else:
                blocks.append((Tb, MODE_LOADMASK))
        pattern.append(tuple(blocks))
    return tuple(pattern)


def _reference_fallback(x, mask, Wq, Wk, Wv, Wo):
    out = np.empty((B, S, H), np.float32)
    for b in range(B):
        q = (x[b] @ Wq).reshape(S, NH, HD).transpose(1, 0, 2)
        k = (x[b] @ Wk).reshape(S, NH, HD).transpose(1, 0, 2)
        v = (x[b] @ Wv).reshape(S, NH, HD).transpose(1, 0, 2)
        s = np.einsum("htd,hTd->htT", q, k) / np.sqrt(HD)
        s = np.where(mask[b][None], s, -1e10)
        s -= s.max(-1, keepdims=True)
        w = np.exp(s)
        w /= w.sum(-1, keepdims=True)
        o = np.einsum("htT,hTd->htd", w, v)
        out[b] = o.transpose(1, 0, 2).reshape(S, NH * HD) @ Wo
    return out


def kernel(x, mask, Wq, Wk, Wv, Wo):
    x = np.asarray(x, np.float32)
    mask = np.asarray(mask).astype(bool)
    Wq = np.asarray(Wq, np.float32)
    Wk = np.asarray(Wk, np.float32)
    Wv = np.asarray(Wv, np.float32)
    Wo = np.asarray(Wo, np.float32)
    assert x.shape == (B, S, H) and mask.shape == (B, S, S)

    pattern = _classify(mask)
    if pattern is None:
        return _reference_fallback(x, mask, Wq, Wk, Wv, Wo)

    if pattern not in _kernel_cache:
        _kernel_cache[pattern] = _build(pattern)
    nc = _kernel_cache[pattern]

    xT_b = [np.ascontiguousarray(x[b].T).astype(_BF16) for b in range(B)]
    maskT_b = [np.ascontiguousarray(mask[b].T).astype(_BF16) for b in range(B)]
    wq_g = [np.ascontiguousarray(Wq[:, g * DPC:(g + 1) * DPC]).astype(_BF16)
            for g in range(GROUPS)]
    wk_g = [np.ascontiguousarray(Wk[:, g * DPC:(g + 1) * DPC]).astype(_BF16)
            for g in range(GROUPS)]
    wv_g = [np.ascontiguousarray(Wv[:, g * DPC:(g + 1) * DPC]).astype(_BF16)
            for g in range(GROUPS)]
    wo_g = [np.ascontiguousarray(Wo[g * DPC:(g + 1) * DPC, :]).astype(_BF16)
            for g in range(GROUPS)]

    in_maps = []
    for i in range(N_CORES):
        b, g = divmod(i, GROUPS)
        in_maps.append({
            "xT": xT_b[b], "maskT": maskT_b[b],
            "wq": wq_g[g], "wk": wk_g[g], "wv": wv_g[g], "wo": wo_g[g],
        })

    from concourse.bass_utils import run_bass_kernel_spmd
    res = run_bass_kernel_spmd(nc, in_maps, core_ids=list(range(N_CORES)))

    out = np.zeros((B, S, H), np.float32)
    for i in range(N_CORES):
        b = i // GROUPS
        out[b] += res.results[i]["out"].astype(np.float32)
    return out
